# revision 6
# baseline (speedup 1.0000x reference)
"""Trainium2 Bass kernel for nn_CRAU (per-channel sparse attention).

Computation (per batch b, channel c):
  qc  = Wq @ src (1x1 conv; bias folded into the S-reduction seed)
  S[c,t] = sum_d unfold(qc)[c,t,d] * feat[c,d] * (1/64)      t in 3x3 window
  A   = softmax_t(S)
  vc  = Wv @ feat + bv (1x1 conv)
  out = fold(A outer vc) * src

Sharding: 8 cores = 4 batches x 2 output-channel halves. The attention is
fully per-channel, so with channel sharding each core is independent (no
collective). Each core needs all 256 input channels of src/feat for the
1x1 convs; the host permutes channels to [own 128 | other 128] so the
SPMD program is core-invariant, and the own-channel block doubles as the
k tensor and the final-multiply src.

Layout: the host packs src as four parity (polyphase) planes of the
padded 129x129 grid in f16; 65-wide planes get a leading zero pad column
(width 66) so every fold-stage block read is 4-byte aligned -- odd-pitch
rows run the DVE/Pool engines at a fraction of peak. The nine q.k taps
run as flat contiguous [128, N] tensor_tensor_reduce ops against a
zero-embedded copy of k (K1 = k at col offset 1 in a 66-wide row; taps
with col offset 2 reuse K1 with the qc run shifted one element, the
boundary products hitting K1's zero pad). Outputs are written as four
64x64 parity planes in f16 and re-interleaved on the host.

Plane layouts (rows x cols, * = zero pad col):
  EE [65 x 66] = [* | P[0::2, 0::2]]   (leading pad)
  EO [65 x 64] =      P[0::2, 1::2]
  OE [64 x 66] = [* | P[1::2, 0::2]]   (leading pad)
  OO [64 x 64] =      P[1::2, 1::2]
where P is the zero-padded [129,129] grid, P[r,c] = x[r-1, c-1].
"""

import numpy as np

N_CORES = 8
SCALE = 1.0 / 64.0

# plane offsets within the packed polyphase layout
P_EE, P_EO, P_OE, P_OO = 0, 4290, 8450, 12674
SRCN = 16770                    # 65*66 + 65*64 + 64*66 + 64*64
FEATN = 4290                    # 65*66 natural padded grid (2 pad cols)
OUTN = 16384                    # 4 planes x 64*64
CH = 2048                       # matmul/copy chunk (PSUM tile) size

_prog_cache = {}
TRACE = False
TRACE_KW = {}
LAST_RESULT = [None]
STAGE = [99]
NATIVE_TTR = [True]


def _build(add_bv: bool, stage: int = 99):
    import concourse.mybir as mybir
    import concourse.tile as tile
    from concourse import bacc
    from concourse.dve_ops import TENSOR_TENSOR_REDUCE

    f32 = mybir.dt.float32
    f16 = mybir.dt.float16
    ADD = mybir.AluOpType.add
    MULT = mybir.AluOpType.mult
    AX = mybir.AxisListType.X
    Exp = mybir.ActivationFunctionType.Exp

    nc = bacc.Bacc("TRN2", target_bir_lowering=False, debug=False,
                   num_devices=N_CORES)

    src_d = nc.dram_tensor("src", [256, SRCN], f16, kind="ExternalInput").ap()
    feat_d = nc.dram_tensor("feat", [256, FEATN], f16, kind="ExternalInput").ap()
    wpack_d = nc.dram_tensor("wpack", [256, 256], f16, kind="ExternalInput").ap()
    sinit_d = nc.dram_tensor("s_init", [128, 9], f32, kind="ExternalInput").ap()
    bv_d = nc.dram_tensor("bv", [128, 1], f32, kind="ExternalInput").ap()
    out_d = nc.dram_tensor("out", [128, OUTN], f16, kind="ExternalOutput").ap()

    n_chunks = (SRCN + CH - 1) // CH            # 9 (last = 386)
    # chunk index after which each plane's qc is fully materialized
    EE_RDY, EO_RDY, OE_RDY, OO_RDY = 2, 4, 6, n_chunks - 1

    with tile.TileContext(nc) as tc:
        with (
            tc.tile_pool(name="constp", bufs=2) as constp,
            tc.tile_pool(name="srcp", bufs=2) as srcp,
            tc.tile_pool(name="featp", bufs=2) as featp,
            tc.tile_pool(name="kp", bufs=1) as kp,
            tc.tile_pool(name="qcp", bufs=1) as qcp,
            tc.tile_pool(name="vcp", bufs=1) as vcp,
            tc.tile_pool(name="smp", bufs=1) as smp,
            tc.tile_pool(name="scrp", bufs=1) as scrp,
            tc.tile_pool(name="tup", bufs=3) as tup,
            tc.tile_pool(name="outp", bufs=2) as outp,
            tc.tile_pool(name="ps", bufs=2, space="PSUM") as ps,
        ):
            # ---- smalls (cols): [0:9] S accum  [9:18] E  [18:27] A
            # [27:28] sumE  [28:29] 1/sumE  [32:41] s_init  [48:49] bv
            sm = smp.tile([128, 64], f32, tag="smalls")
            nc.sync.dma_start(sm[:, 32:41], sinit_d[:, :])
            if add_bv:
                nc.sync.dma_start(sm[:, 48:49], bv_d[:, :])

            # ---- loads: weights, then src chunks (kt-interleaved), feat ----
            w_t = []
            for kt in range(2):
                wt = constp.tile([128, 256], f16, tag="w")
                nc.sync.dma_start(wt[:], wpack_d[128 * kt:128 * kt + 128, :])
                w_t.append(wt)
            src_t = [srcp.tile([128, SRCN], f16, tag="src", name=f"src{k}")
                     for k in range(2)]
            feat_t = [featp.tile([128, FEATN], f16, tag="feat",
                                 name=f"feat{k}") for k in range(2)]

            # k embeddings: K1 = k at col offset 1 in 66-wide rows (zeros
            # in cols 0 and 65); k64 = packed 64-wide k. Built with
            # SBUF->SBUF DMA off the engines' critical path.
            ktile = kp.tile([128, 8320], f16, tag="K")
            K1f = ktile[:, 0:4224]
            k64f = ktile[:, 4224:8320]
            K1v = K1f.rearrange("p (r q) -> p r q", q=66)
            nc.gpsimd.memset(K1v[:, :, 0:1], 0.0)
            nc.gpsimd.memset(K1v[:, :, 65:66], 0.0)

            for c in range(n_chunks):
                c0 = CH * c
                csz = min(CH, SRCN - c0)
                for kt in range(2):
                    nc.sync.dma_start(
                        src_t[kt][:, c0:c0 + csz],
                        src_d[128 * kt:128 * kt + 128, c0:c0 + csz])
                if c == 0:                      # k needed by the first taps
                    nc.sync.dma_start(feat_t[0][:], feat_d[0:128, :])
                    ksrc = feat_t[0].rearrange(
                        "p (r q) -> p r q", q=66)[:, 0:64, 0:64]
                    nc.sync.dma_start(K1v[:, :, 1:65], ksrc)
                    nc.sync.dma_start(
                        k64f.rearrange("p (r q) -> p r q", q=64)[:], ksrc)
                if c == 2:                      # v-conv input, needed later
                    nc.sync.dma_start(feat_t[1][:], feat_d[128:256, :])

            qc = qcp.tile([128, SRCN], f16, tag="qc")

            # taps: (tap_idx, qc flat range start, length, k embedding)
            # col-offset-2 taps ride K1 with the qc run shifted +1 elem;
            # the two boundary products hit K1's zero pad columns.
            tap_sched = {
                EE_RDY: [(0, P_EE, 4224, K1f), (2, P_EE + 1, 4224, K1f),
                         (6, P_EE + 66, 4224, K1f),
                         (8, P_EE + 67, 4224, K1f)],
                EO_RDY: [(1, P_EO, 4096, k64f), (7, P_EO + 64, 4096, k64f)],
                OE_RDY: [(3, P_OE, 4224, K1f), (5, P_OE + 1, 4224, K1f)],
                OO_RDY: [(4, P_OO, 4096, k64f)],
            }

            def emit_tap(t, q0, qlen, kf):
                scr = scrp.tile([128, 4224], f16, tag="scr")
                if NATIVE_TTR[0]:
                    nc.vector.tensor_tensor_reduce(
                        out=scr[:, 0:qlen], in0=qc[:, q0:q0 + qlen],
                        in1=kf[:, 0:qlen], scale=SCALE,
                        scalar=sm[:, 32 + t:33 + t], op0=MULT, op1=ADD,
                        accum_out=sm[:, t:t + 1])
                else:
                    nc.vector._custom_dve(
                        TENSOR_TENSOR_REDUCE,
                        out=scr[:, 0:qlen], in0=qc[:, q0:q0 + qlen],
                        in1=kf[:, 0:qlen],
                        s0=sm[:, 32 + t:33 + t], s1=SCALE,
                        accum_out=sm[:, t:t + 1])

            def emit_vconv():
                vc = vcp.tile([128, FEATN], f16, tag="vc")
                for c0 in range(0, FEATN, CH):
                    csz = min(CH, FEATN - c0)
                    pt = ps.tile([128, CH], f32, tag="mm")
                    for kt in range(2):
                        for s0 in range(0, csz, 512):
                            ssz = min(512, csz - s0)
                            nc.tensor.matmul(
                                pt[:, s0:s0 + ssz],
                                lhsT=w_t[kt][:, 128:256],
                                rhs=feat_t[kt][:, c0 + s0:c0 + s0 + ssz],
                                start=(kt == 0), stop=(kt == 1))
                    if add_bv:
                        nc.vector.tensor_scalar(
                            out=vc[:, c0:c0 + csz], in0=pt[:, 0:csz],
                            scalar1=sm[:, 48:49], scalar2=None, op0=ADD)
                    else:
                        nc.scalar.copy(vc[:, c0:c0 + csz], pt[:, 0:csz])
                vc3 = vc.rearrange("p (r q) -> p r q", q=66)
                if add_bv:      # re-zero the padded row/cols polluted by +bv
                    nc.gpsimd.memset(vc3[:, 64, :], 0.0)
                    nc.gpsimd.memset(vc3[:, :, 64:66], 0.0)
                # aligned shadow of the col-shifted vc (for v01 / v11),
                # built by SBUF->SBUF DMA
                vcs = vcp.tile([128, 65 * 64], f16, tag="vcs")
                vcs3 = vcs.rearrange("p (r q) -> p r q", q=64)
                nc.sync.dma_start(vcs3[:], vc3[:, 0:65, 1:65])
                return vc3, vcs3

            # ---- q-conv + chunk copies + taps, pipelined; v-conv is
            # hoisted before the tiny last q-chunk so vc is ready when
            # the final tap + softmax complete ----
            vc3 = vcs3 = None
            for c in range(n_chunks):
                if c == n_chunks - 1 and stage >= 4:
                    vc3, vcs3 = emit_vconv()
                c0 = CH * c
                csz = min(CH, SRCN - c0)
                pt = ps.tile([128, CH], f32, tag="mm")
                for kt in range(2):
                    for s0 in range(0, csz, 512):
                        ssz = min(512, csz - s0)
                        nc.tensor.matmul(
                            pt[:, s0:s0 + ssz],
                            lhsT=w_t[kt][:, 0:128],
                            rhs=src_t[kt][:, c0 + s0:c0 + s0 + ssz],
                            start=(kt == 0), stop=(kt == 1))
                if stage < 2 and c > 0:
                    continue
                nc.scalar.copy(qc[:, c0:c0 + csz], pt[:, 0:csz])
                if stage >= 2:
                    for args in tap_sched.get(c, []):
                        emit_tap(*args)

            if stage == 2:
                nc.sync.dma_start(out_d[:, 0:9], sm[:, 0:9])

            # ---- softmax over the 9 taps (no max-sub: |logit| <~ 8) ----
            if stage >= 3:
                nc.scalar.activation(sm[:, 9:18], sm[:, 0:9], Exp,
                                     bias=0.0, scale=1.0)
                nc.vector.tensor_reduce(sm[:, 27:28], sm[:, 9:18],
                                        axis=AX, op=ADD)
                nc.vector.reciprocal(sm[:, 28:29], sm[:, 27:28])
                nc.vector.tensor_scalar(out=sm[:, 18:27], in0=sm[:, 9:18],
                                        scalar1=sm[:, 28:29], scalar2=None,
                                        op0=MULT)
            if stage == 3:
                nc.sync.dma_start(out_d[:, 16:25], sm[:, 18:27])

            # ---- fold + final src multiply, one parity plane at a time ----
            if stage >= 5:
                def a(t):
                    return sm[:, 18 + t:19 + t]

                v00 = vc3[:, 0:64, 0:64]
                v10 = vc3[:, 1:65, 0:64]
                v01 = vcs3[:, 0:64, :]
                v11 = vcs3[:, 1:65, :]
                s3 = src_t[0]
                sEE = s3[:, P_EE:P_EO].rearrange("p (r q) -> p r q", q=66)
                sEO = s3[:, P_EO:P_OE].rearrange("p (r q) -> p r q", q=64)
                sOE = s3[:, P_OE:P_OO].rearrange("p (r q) -> p r q", q=66)
                sOO = s3[:, P_OO:SRCN].rearrange("p (r q) -> p r q", q=64)

                def v2(tl):
                    return tl.rearrange("p (r q) -> p r q", q=64)

                # ee: (A4*v00) * src  -- single fused op, DMA out first
                Pee = outp.tile([128, 4096], f16, tag="O")
                nc.vector.scalar_tensor_tensor(
                    out=v2(Pee), in0=v00, scalar=a(4), in1=sOO[:, 0:64, 0:64],
                    op0=MULT, op1=MULT)
                nc.sync.dma_start(out_d[:, 0:4096], Pee[:])

                # oo: (A0*v11 + A2*v10 + A6*v01 + A8*v00) * src
                T3 = tup.tile([128, 4096], f16, tag="tu")
                nc.scalar.mul(v2(T3), v11, a(0))
                T4 = tup.tile([128, 4096], f16, tag="tu")
                nc.gpsimd.tensor_scalar(out=v2(T4), in0=v01, scalar1=a(6),
                                        scalar2=None, op0=MULT)
                U3 = tup.tile([128, 4096], f16, tag="tu")
                nc.vector.scalar_tensor_tensor(
                    out=v2(U3), in0=v10, scalar=a(2), in1=v2(T3),
                    op0=MULT, op1=ADD)
                U4 = tup.tile([128, 4096], f16, tag="tu")
                nc.vector.scalar_tensor_tensor(
                    out=v2(U4), in0=v00, scalar=a(8), in1=v2(T4),
                    op0=MULT, op1=ADD)
                U5 = tup.tile([128, 4096], f16, tag="tu")
                nc.vector.tensor_tensor(out=v2(U5), in0=v2(U3), in1=v2(U4),
                                        op=ADD)
                Poo = outp.tile([128, 4096], f16, tag="O")
                nc.gpsimd.tensor_tensor(out=v2(Poo), in0=v2(U5),
                                        in1=sEE[:, 1:65, 2:66], op=MULT)
                nc.sync.dma_start(out_d[:, 12288:16384], Poo[:])

                # eo: (A3*v01 + A5*v00) * src
                T1 = tup.tile([128, 4096], f16, tag="tu")
                nc.scalar.mul(v2(T1), v00, a(5))
                U1 = tup.tile([128, 4096], f16, tag="tu")
                nc.vector.scalar_tensor_tensor(
                    out=v2(U1), in0=v01, scalar=a(3), in1=v2(T1),
                    op0=MULT, op1=ADD)
                Peo = outp.tile([128, 4096], f16, tag="O")
                nc.gpsimd.tensor_tensor(out=v2(Peo), in0=v2(U1),
                                        in1=sOE[:, 0:64, 2:66], op=MULT)
                nc.sync.dma_start(out_d[:, 4096:8192], Peo[:])

                # oe: (A1*v10 + A7*v00) * src
                T2 = tup.tile([128, 4096], f16, tag="tu")
                nc.gpsimd.tensor_scalar(out=v2(T2), in0=v10, scalar1=a(1),
                                        scalar2=None, op0=MULT)
                U2 = tup.tile([128, 4096], f16, tag="tu")
                nc.vector.scalar_tensor_tensor(
                    out=v2(U2), in0=v00, scalar=a(7), in1=v2(T2),
                    op0=MULT, op1=ADD)
                Poe = outp.tile([128, 4096], f16, tag="O")
                nc.gpsimd.tensor_tensor(out=v2(Poe), in0=v2(U2),
                                        in1=sEO[:, 1:65, 0:64], op=MULT)
                nc.sync.dma_start(out_d[:, 8192:12288], Poe[:])

    nc.compile()
    return nc


def _get_program(add_bv: bool, stage: int = 99):
    key = (add_bv, stage, NATIVE_TTR[0])
    if key not in _prog_cache:
        _prog_cache[key] = _build(add_bv, stage)
    return _prog_cache[key]


def _polyphase(x):
    # x: [B, C, 129, 129] padded f16 -> [B, C, 16770] plane-packed with
    # leading zero pad col on the 65-wide (even-col) planes
    B, C = x.shape[:2]
    ee = np.zeros((B, C, 65, 66), np.float16)
    ee[:, :, :, 1:66] = x[:, :, 0::2, 0::2]
    oe = np.zeros((B, C, 64, 66), np.float16)
    oe[:, :, :, 1:66] = x[:, :, 1::2, 0::2]
    return np.concatenate([
        ee.reshape(B, C, -1),
        x[:, :, 0::2, 1::2].reshape(B, C, -1),
        oe.reshape(B, C, -1),
        x[:, :, 1::2, 1::2].reshape(B, C, -1),
    ], axis=2)


def kernel(feat, src, Wq, bq, Wv, bv):
    from concourse.bass_utils import run_bass_kernel_spmd

    feat = np.asarray(feat, dtype=np.float32)
    src = np.asarray(src, dtype=np.float32)
    Wq = np.asarray(Wq, dtype=np.float32)
    bq = np.asarray(bq, dtype=np.float32)
    Wv = np.asarray(Wv, dtype=np.float32)
    bv = np.asarray(bv, dtype=np.float32)
    B, C, H, W = src.shape
    CH_HALF = C // 2

    # padded 129x129 grid (row/col -1 pad; right/bottom pad never read)
    src_pad = np.zeros((B, C, 129, 129), np.float16)
    src_pad[:, :, 1:129, 1:129] = src
    src_pk = _polyphase(src_pad)                       # [B, C, 16770]
    feat_pk = np.zeros((B, C, 65, 66), np.float16)
    feat_pk[:, :, 0:64, 0:64] = feat
    feat_pk = feat_pk.reshape(B, C, FEATN)

    add_bv = bool(np.any(bv))
    nc = _get_program(add_bv, STAGE[0])

    in_maps = []
    for core in range(N_CORES):
        b, u = core // 2, core % 2
        own = slice(CH_HALF * u, CH_HALF * u + CH_HALF)
        perm = np.r_[own, slice(CH_HALF * (1 - u), CH_HALF * (1 - u) + CH_HALF)]
        wpack = np.concatenate(
            [Wq[own][:, perm].T, Wv[own][:, perm].T], axis=1
        ).astype(np.float16)
        # bq correction seed: S += bq * sum(valid k) * scale; valid excludes
        # x=0 when i==0 and y=0 when j==0 (qc zero-pad positions).
        if np.any(bq):
            k = feat[b, own].astype(np.float64)
            tot = k.sum((1, 2))
            no_r0 = tot - k[:, 0, :].sum(1)
            no_c0 = tot - k[:, :, 0].sum(1)
            no_rc = no_r0 - k[:, :, 0].sum(1) + k[:, 0, 0]
            sums = [no_rc, no_r0, no_r0, no_c0, tot, tot, no_c0, tot, tot]
            sinit = (np.stack(sums, 1) * bq[own, None] * SCALE).astype(
                np.float32)
        else:
            sinit = np.zeros((CH_HALF, 9), np.float32)
        in_maps.append({
            "src": np.ascontiguousarray(src_pk[b, perm]),
            "feat": np.ascontiguousarray(feat_pk[b, perm]),
            "wpack": np.ascontiguousarray(wpack),
            "s_init": sinit,
            "bv": bv[own].reshape(CH_HALF, 1).astype(np.float32),
        })

    res = run_bass_kernel_spmd(nc, in_maps, list(range(N_CORES)),
                               trace=TRACE, **TRACE_KW)
    LAST_RESULT[0] = res

    out = np.empty((B, C, H, W), np.float32)
    for core in range(N_CORES):
        b, u = core // 2, core % 2
        own = slice(CH_HALF * u, CH_HALF * u + CH_HALF)
        r = res.results[core]["out"].astype(np.float32).reshape(
            CH_HALF, 4, 64, 64)
        out[b, own, 0::2, 0::2] = r[:, 0]
        out[b, own, 0::2, 1::2] = r[:, 1]
        out[b, own, 1::2, 0::2] = r[:, 2]
        out[b, own, 1::2, 1::2] = r[:, 3]
    return out


# revision 9
# speedup vs baseline: 1.4475x; 1.4475x over previous
"""Trainium2 Bass kernel for nn_CRAU (per-channel sparse attention).

Computation (per batch b, channel c):
  qc  = Wq @ src (1x1 conv; bias folded into the S-reduction seed)
  S[c,t] = sum_d unfold(qc)[c,t,d] * feat[c,d] * (1/64)      t in 3x3 window
  A   = softmax_t(S)
  vc  = Wv @ feat + bv (1x1 conv)
  out = fold(A outer vc) * src

Sharding: 8 cores = 4 batches x 2 output-channel halves. The attention is
fully per-channel, so with channel sharding each core is independent (no
collective). Each core needs all 256 input channels of src/feat for the
1x1 convs; the host permutes channels to [own 128 | other 128] so the
SPMD program is core-invariant, and the own-channel block doubles as the
k tensor and the final-multiply src.

Layout: the host packs src as four parity (polyphase) planes of the
padded 129x129 grid in f16; 65-wide planes get a leading zero pad column
(width 66) so every fold-stage block read is 4-byte aligned. Each q.k
tap is a flat [128, N] f16 tensor_tensor product (2x DVE mode) followed
by a tensor_scalar pass with accum_out (4x DVE mode) that applies the
1/64 scale and reduces -- the dedicated reduce ops are capped at 1
elem/cycle. Taps with window col offset 2 reuse the col-offset-1 zero
embedded k (K1) with the qc run shifted one element; boundary products
land on K1's zero pad columns. Outputs are four 64x64 parity planes in
f16, re-interleaved on the host.

Plane layouts (rows x cols, * = zero pad col):
  EE [65 x 66] = [* | P[0::2, 0::2]]   (leading pad)
  EO [65 x 64] =      P[0::2, 1::2]
  OE [64 x 66] = [* | P[1::2, 0::2]]   (leading pad)
  OO [64 x 64] =      P[1::2, 1::2]
where P is the zero-padded [129,129] grid, P[r,c] = x[r-1, c-1].
"""

import numpy as np

N_CORES = 8
SCALE = 1.0 / 64.0

# plane offsets within the packed polyphase layout
P_EE, P_EO, P_OE, P_OO = 0, 4290, 8450, 12674
SRCN = 16770                    # 65*66 + 65*64 + 64*66 + 64*64
FEATN = 4290                    # 65*66 natural padded grid (2 pad cols)
OUTN = 16384                    # 4 planes x 64*64
CH = 2048                       # matmul/copy chunk (PSUM tile) size

_prog_cache = {}
TRACE = False
TRACE_KW = {}
LAST_RESULT = [None]
STAGE = [99]


def _build(add_bv: bool, stage: int = 99):
    import concourse.mybir as mybir
    import concourse.tile as tile
    from concourse import bacc

    f32 = mybir.dt.float32
    f16 = mybir.dt.float16
    ADD = mybir.AluOpType.add
    MULT = mybir.AluOpType.mult
    AX = mybir.AxisListType.X
    Exp = mybir.ActivationFunctionType.Exp

    nc = bacc.Bacc("TRN2", target_bir_lowering=False, debug=False,
                   num_devices=N_CORES)

    src_d = nc.dram_tensor("src", [256, SRCN], f16, kind="ExternalInput").ap()
    feat_d = nc.dram_tensor("feat", [256, FEATN], f16, kind="ExternalInput").ap()
    wpack_d = nc.dram_tensor("wpack", [256, 256], f16, kind="ExternalInput").ap()
    sinit_d = nc.dram_tensor("s_init", [128, 9], f32, kind="ExternalInput").ap()
    bv_d = nc.dram_tensor("bv", [128, 1], f32, kind="ExternalInput").ap()
    out_d = nc.dram_tensor("out", [128, OUTN], f16, kind="ExternalOutput").ap()

    n_chunks = (SRCN + CH - 1) // CH            # 9 (last = 386)
    # chunk index after which each plane's qc is fully materialized
    EE_RDY, EO_RDY, OE_RDY, OO_RDY = 2, 4, 6, n_chunks - 1

    with tile.TileContext(nc) as tc:
        with (
            tc.tile_pool(name="constp", bufs=2) as constp,
            tc.tile_pool(name="srcp", bufs=2) as srcp,
            tc.tile_pool(name="featp", bufs=2) as featp,
            tc.tile_pool(name="kp", bufs=1) as kp,
            tc.tile_pool(name="qcp", bufs=1) as qcp,
            tc.tile_pool(name="vcp", bufs=1) as vcp,
            tc.tile_pool(name="smp", bufs=1) as smp,
            tc.tile_pool(name="scrp", bufs=2) as scrp,
            tc.tile_pool(name="tup", bufs=1) as tup,
            tc.tile_pool(name="outp", bufs=1) as outp,
            tc.tile_pool(name="ps", bufs=2, space="PSUM") as ps,
        ):
            # ---- smalls (cols): [0:9] S accum  [9:18] E  [18:27] A
            # [27:28] sumE  [28:29] 1/sumE  [32:41] s_init  [48:49] bv
            sm = smp.tile([128, 64], f32, tag="smalls")
            nc.sync.dma_start(sm[:, 32:41], sinit_d[:, :])
            if add_bv:
                nc.sync.dma_start(sm[:, 48:49], bv_d[:, :])

            # ---- loads: weights, then src chunks (kt-interleaved), feat ----
            w_t = []
            for kt in range(2):
                wt = constp.tile([128, 256], f16, tag="w")
                nc.sync.dma_start(wt[:], wpack_d[128 * kt:128 * kt + 128, :])
                w_t.append(wt)
            src_t = [srcp.tile([128, SRCN], f16, tag="src", name=f"src{k}")
                     for k in range(2)]
            feat_t = [featp.tile([128, FEATN], f16, tag="feat",
                                 name=f"feat{k}") for k in range(2)]

            # k embeddings: K1 = k at col offset 1 in 66-wide rows (zeros
            # in cols 0 and 65); k64 = packed 64-wide k. Built with
            # SBUF->SBUF DMA off the engines' critical path.
            ktile = kp.tile([128, 8320], f16, tag="K")
            K1f = ktile[:, 0:4224]
            k64f = ktile[:, 4224:8320]
            K1v = K1f.rearrange("p (r q) -> p r q", q=66)
            nc.gpsimd.memset(K1v[:, :, 0:1], 0.0)
            nc.gpsimd.memset(K1v[:, :, 65:66], 0.0)

            for c in range(n_chunks):
                c0 = CH * c
                csz = min(CH, SRCN - c0)
                for kt in range(2):
                    nc.sync.dma_start(
                        src_t[kt][:, c0:c0 + csz],
                        src_d[128 * kt:128 * kt + 128, c0:c0 + csz])
                if c == 0:                      # k needed by the first taps
                    nc.sync.dma_start(feat_t[0][:], feat_d[0:128, :])
                    ksrc = feat_t[0].rearrange(
                        "p (r q) -> p r q", q=66)[:, 0:64, 0:64]
                    nc.sync.dma_start(K1v[:, :, 1:65], ksrc)
                    nc.sync.dma_start(
                        k64f.rearrange("p (r q) -> p r q", q=64)[:], ksrc)
                if c == 2:                      # v-conv input, needed later
                    nc.sync.dma_start(feat_t[1][:], feat_d[128:256, :])

            qc = qcp.tile([128, SRCN], f16, tag="qc")

            # taps: (tap_idx, qc flat range start, length, k embedding)
            tap_sched = {
                EE_RDY: [(0, P_EE, 4224, K1f), (2, P_EE + 1, 4224, K1f),
                         (6, P_EE + 66, 4224, K1f),
                         (8, P_EE + 67, 4224, K1f)],
                EO_RDY: [(1, P_EO, 4096, k64f), (7, P_EO + 64, 4096, k64f)],
                OE_RDY: [(3, P_OE, 4224, K1f), (5, P_OE + 1, 4224, K1f)],
                OO_RDY: [(4, P_OO, 4096, k64f)],
            }

            def emit_tap(t, q0, qlen, kf):
                # product at DVE 2x, then scale+reduce at DVE 4x (accum_out)
                prod = scrp.tile([128, 4224], f16, tag="scr", name=f"pr{t}")
                nc.vector.tensor_tensor(out=prod[:, 0:qlen],
                                        in0=qc[:, q0:q0 + qlen],
                                        in1=kf[:, 0:qlen], op=MULT)
                red = scrp.tile([128, 4224], f16, tag="scr", name=f"rd{t}")
                nc.vector.tensor_scalar(out=red[:, 0:qlen],
                                        in0=prod[:, 0:qlen],
                                        scalar1=SCALE, scalar2=None,
                                        op0=MULT, op1=ADD,
                                        accum_out=sm[:, t:t + 1])

            def emit_vconv():
                vc = vcp.tile([128, FEATN], f16, tag="vc")
                for c0 in range(0, FEATN, CH):
                    csz = min(CH, FEATN - c0)
                    pt = ps.tile([128, CH], f32, tag="mm")
                    for kt in range(2):
                        for s0 in range(0, csz, 512):
                            ssz = min(512, csz - s0)
                            nc.tensor.matmul(
                                pt[:, s0:s0 + ssz],
                                lhsT=w_t[kt][:, 128:256],
                                rhs=feat_t[kt][:, c0 + s0:c0 + s0 + ssz],
                                start=(kt == 0), stop=(kt == 1))
                    if add_bv:
                        nc.vector.tensor_scalar(
                            out=vc[:, c0:c0 + csz], in0=pt[:, 0:csz],
                            scalar1=sm[:, 48:49], scalar2=None, op0=ADD)
                    else:
                        nc.scalar.copy(vc[:, c0:c0 + csz], pt[:, 0:csz])
                vc3 = vc.rearrange("p (r q) -> p r q", q=66)
                if add_bv:      # re-zero the padded row/cols polluted by +bv
                    nc.gpsimd.memset(vc3[:, 64, :], 0.0)
                    nc.gpsimd.memset(vc3[:, :, 64:66], 0.0)
                # aligned shadow of the col-shifted vc (for v01 / v11),
                # built by SBUF->SBUF DMA
                vcs = vcp.tile([128, 65 * 64], f16, tag="vcs")
                vcs3 = vcs.rearrange("p (r q) -> p r q", q=64)
                nc.sync.dma_start(vcs3[:], vc3[:, 0:65, 1:65])
                return vc3, vcs3

            # ---- q-conv + chunk copies + taps, pipelined; v-conv is
            # hoisted before the tiny last q-chunk so vc is ready when
            # the final tap + softmax complete ----
            vc3 = vcs3 = None
            for c in range(n_chunks):
                if c == n_chunks - 1 and stage >= 4:
                    vc3, vcs3 = emit_vconv()
                c0 = CH * c
                csz = min(CH, SRCN - c0)
                pt = ps.tile([128, CH], f32, tag="mm")
                for kt in range(2):
                    for s0 in range(0, csz, 512):
                        ssz = min(512, csz - s0)
                        nc.tensor.matmul(
                            pt[:, s0:s0 + ssz],
                            lhsT=w_t[kt][:, 0:128],
                            rhs=src_t[kt][:, c0 + s0:c0 + s0 + ssz],
                            start=(kt == 0), stop=(kt == 1))
                if stage < 2 and c > 0:
                    continue
                nc.scalar.copy(qc[:, c0:c0 + csz], pt[:, 0:csz])
                if stage >= 2:
                    for args in tap_sched.get(c, []):
                        emit_tap(*args)

            # fold in the bq-correction seeds (zeros when bq == 0)
            if stage >= 2:
                nc.vector.tensor_tensor(out=sm[:, 0:9], in0=sm[:, 0:9],
                                        in1=sm[:, 32:41], op=ADD)
            if stage == 2:
                nc.sync.dma_start(out_d[:, 0:9], sm[:, 0:9])

            # ---- softmax over the 9 taps (no max-sub: |logit| <~ 8) ----
            if stage >= 3:
                nc.scalar.activation(sm[:, 9:18], sm[:, 0:9], Exp,
                                     bias=0.0, scale=1.0)
                nc.vector.tensor_reduce(sm[:, 27:28], sm[:, 9:18],
                                        axis=AX, op=ADD)
                nc.vector.reciprocal(sm[:, 28:29], sm[:, 27:28])
                nc.vector.tensor_scalar(out=sm[:, 18:27], in0=sm[:, 9:18],
                                        scalar1=sm[:, 28:29], scalar2=None,
                                        op0=MULT)
            if stage == 3:
                nc.sync.dma_start(out_d[:, 16:25], sm[:, 18:27])

            # ---- fold + final src multiply, one parity plane at a time ----
            # Fixed-tag working tiles (A..D) sequence the mul/add tree
            # explicitly; rotation pools can't express this dependency
            # pattern safely.
            if stage >= 5:
                def a(t):
                    return sm[:, 18 + t:19 + t]

                v00 = vc3[:, 0:64, 0:64]
                v10 = vc3[:, 1:65, 0:64]
                v01 = vcs3[:, 0:64, :]
                v11 = vcs3[:, 1:65, :]
                s3 = src_t[0]
                sEE = s3[:, P_EE:P_EO].rearrange("p (r q) -> p r q", q=66)
                sEO = s3[:, P_EO:P_OE].rearrange("p (r q) -> p r q", q=64)
                sOE = s3[:, P_OE:P_OO].rearrange("p (r q) -> p r q", q=66)
                sOO = s3[:, P_OO:SRCN].rearrange("p (r q) -> p r q", q=64)

                def v2(tl):
                    return tl.rearrange("p (r q) -> p r q", q=64)

                tA = tup.tile([128, 4096], f16, tag="tuA")
                tB = tup.tile([128, 4096], f16, tag="tuB")
                tC = tup.tile([128, 4096], f16, tag="tuC")
                tD = tup.tile([128, 4096], f16, tag="tuD")

                # oo: (A0*v11 + A2*v10 + A6*v01 + A8*v00) * src
                nc.vector.tensor_scalar(out=v2(tA), in0=v11, scalar1=a(0),
                                        scalar2=None, op0=MULT)
                nc.vector.tensor_scalar(out=v2(tB), in0=v10, scalar1=a(2),
                                        scalar2=None, op0=MULT)
                nc.scalar.mul(v2(tC), v01, a(6))
                nc.scalar.mul(v2(tD), v00, a(8))
                nc.vector.tensor_tensor(out=v2(tA), in0=v2(tA), in1=v2(tB),
                                        op=ADD)
                nc.vector.tensor_tensor(out=v2(tC), in0=v2(tC), in1=v2(tD),
                                        op=ADD)
                nc.vector.tensor_tensor(out=v2(tA), in0=v2(tA), in1=v2(tC),
                                        op=ADD)
                Poo = outp.tile([128, 4096], f16, tag="O")
                nc.vector.tensor_tensor(out=v2(Poo), in0=v2(tA),
                                        in1=sEE[:, 1:65, 2:66], op=MULT)
                nc.sync.dma_start(out_d[:, 12288:16384], Poo[:])

                # ee: (A4*v00) * src
                nc.vector.tensor_scalar(out=v2(tB), in0=v00, scalar1=a(4),
                                        scalar2=None, op0=MULT)
                Pee = outp.tile([128, 4096], f16, tag="O")
                nc.vector.tensor_tensor(out=v2(Pee), in0=v2(tB),
                                        in1=sOO[:, 0:64, 0:64], op=MULT)
                nc.sync.dma_start(out_d[:, 0:4096], Pee[:])

                # eo: (A3*v01 + A5*v00) * src
                nc.vector.tensor_scalar(out=v2(tD), in0=v01, scalar1=a(3),
                                        scalar2=None, op0=MULT)
                nc.scalar.mul(v2(tC), v00, a(5))
                nc.vector.tensor_tensor(out=v2(tD), in0=v2(tD), in1=v2(tC),
                                        op=ADD)
                Peo = outp.tile([128, 4096], f16, tag="O")
                nc.gpsimd.tensor_tensor(out=v2(Peo), in0=v2(tD),
                                        in1=sOE[:, 0:64, 2:66], op=MULT)
                nc.sync.dma_start(out_d[:, 4096:8192], Peo[:])

                # oe: (A1*v10 + A7*v00) * src
                nc.vector.tensor_scalar(out=v2(tA), in0=v10, scalar1=a(1),
                                        scalar2=None, op0=MULT)
                nc.scalar.mul(v2(tB), v00, a(7))
                nc.vector.tensor_tensor(out=v2(tA), in0=v2(tA), in1=v2(tB),
                                        op=ADD)
                Poe = outp.tile([128, 4096], f16, tag="O")
                nc.gpsimd.tensor_tensor(out=v2(Poe), in0=v2(tA),
                                        in1=sEO[:, 1:65, 0:64], op=MULT)
                nc.sync.dma_start(out_d[:, 8192:12288], Poe[:])

    nc.compile()
    return nc


def _get_program(add_bv: bool, stage: int = 99):
    key = (add_bv, stage)
    if key not in _prog_cache:
        _prog_cache[key] = _build(add_bv, stage)
    return _prog_cache[key]


def _polyphase(x):
    # x: [B, C, 129, 129] padded f16 -> [B, C, 16770] plane-packed with
    # leading zero pad col on the 65-wide (even-col) planes
    B, C = x.shape[:2]
    ee = np.zeros((B, C, 65, 66), np.float16)
    ee[:, :, :, 1:66] = x[:, :, 0::2, 0::2]
    oe = np.zeros((B, C, 64, 66), np.float16)
    oe[:, :, :, 1:66] = x[:, :, 1::2, 0::2]
    return np.concatenate([
        ee.reshape(B, C, -1),
        x[:, :, 0::2, 1::2].reshape(B, C, -1),
        oe.reshape(B, C, -1),
        x[:, :, 1::2, 1::2].reshape(B, C, -1),
    ], axis=2)


def kernel(feat, src, Wq, bq, Wv, bv):
    from concourse.bass_utils import run_bass_kernel_spmd

    feat = np.asarray(feat, dtype=np.float32)
    src = np.asarray(src, dtype=np.float32)
    Wq = np.asarray(Wq, dtype=np.float32)
    bq = np.asarray(bq, dtype=np.float32)
    Wv = np.asarray(Wv, dtype=np.float32)
    bv = np.asarray(bv, dtype=np.float32)
    B, C, H, W = src.shape
    CH_HALF = C // 2

    # padded 129x129 grid (row/col -1 pad; right/bottom pad never read)
    src_pad = np.zeros((B, C, 129, 129), np.float16)
    src_pad[:, :, 1:129, 1:129] = src
    src_pk = _polyphase(src_pad)                       # [B, C, 16770]
    feat_pk = np.zeros((B, C, 65, 66), np.float16)
    feat_pk[:, :, 0:64, 0:64] = feat
    feat_pk = feat_pk.reshape(B, C, FEATN)

    add_bv = bool(np.any(bv))
    nc = _get_program(add_bv, STAGE[0])

    in_maps = []
    for core in range(N_CORES):
        b, u = core // 2, core % 2
        own = slice(CH_HALF * u, CH_HALF * u + CH_HALF)
        perm = np.r_[own, slice(CH_HALF * (1 - u), CH_HALF * (1 - u) + CH_HALF)]
        wpack = np.concatenate(
            [Wq[own][:, perm].T, Wv[own][:, perm].T], axis=1
        ).astype(np.float16)
        # bq correction seed: S += bq * sum(valid k) * scale; valid excludes
        # x=0 when i==0 and y=0 when j==0 (qc zero-pad positions).
        if np.any(bq):
            k = feat[b, own].astype(np.float64)
            tot = k.sum((1, 2))
            no_r0 = tot - k[:, 0, :].sum(1)
            no_c0 = tot - k[:, :, 0].sum(1)
            no_rc = no_r0 - k[:, :, 0].sum(1) + k[:, 0, 0]
            sums = [no_rc, no_r0, no_r0, no_c0, tot, tot, no_c0, tot, tot]
            sinit = (np.stack(sums, 1) * bq[own, None] * SCALE).astype(
                np.float32)
        else:
            sinit = np.zeros((CH_HALF, 9), np.float32)
        in_maps.append({
            "src": np.ascontiguousarray(src_pk[b, perm]),
            "feat": np.ascontiguousarray(feat_pk[b, perm]),
            "wpack": np.ascontiguousarray(wpack),
            "s_init": sinit,
            "bv": bv[own].reshape(CH_HALF, 1).astype(np.float32),
        })

    res = run_bass_kernel_spmd(nc, in_maps, list(range(N_CORES)),
                               trace=TRACE, **TRACE_KW)
    LAST_RESULT[0] = res

    out = np.empty((B, C, H, W), np.float32)
    for core in range(N_CORES):
        b, u = core // 2, core % 2
        own = slice(CH_HALF * u, CH_HALF * u + CH_HALF)
        r = res.results[core]["out"].astype(np.float32).reshape(
            CH_HALF, 4, 64, 64)
        out[b, own, 0::2, 0::2] = r[:, 0]
        out[b, own, 0::2, 1::2] = r[:, 1]
        out[b, own, 1::2, 0::2] = r[:, 2]
        out[b, own, 1::2, 1::2] = r[:, 3]
    return out


# revision 11
# speedup vs baseline: 1.8576x; 1.2833x over previous
"""Trainium2 Bass kernel for nn_CRAU (per-channel sparse attention).

Computation (per batch b, channel c):
  qc  = Wq @ src (1x1 conv; bias folded into the exp bias)
  S[c,t] = sum_d unfold(qc)[c,t,d] * feat[c,d] * (1/64)      t in 3x3 window
  A   = softmax_t(S)
  vc  = Wv @ feat + bv (1x1 conv)
  out = fold(A outer vc) * src

Sharding: 8 cores = 4 batches x 2 output-channel halves. The attention is
fully per-channel, so with channel sharding each core is independent (no
collective). Each core needs all 256 input channels of src/feat for the
1x1 convs; the host permutes channels to [own 128 | other 128] so the
SPMD program is core-invariant, and the own-channel block doubles as the
k tensor and the final-multiply src.

Measured DVE rates (f16, aligned): tensor_scalar 4x (0.26 ns/elem),
tensor_tensor 2x (0.52), tensor_tensor_reduce / tensor_scalar+accum 1x
(1.07). Engine split per core:
 - 3 window taps run as single custom TENSOR_TENSOR_REDUCE ops on Vector;
   6 run as Vector tensor_tensor products + Scalar activation(Copy,
   scale=1/64, accum_out) reductions, balancing Vector vs Scalar.
 - exp(S_t + s_init_t) runs per tap on Scalar as soon as that tap lands;
   the unnormalized fold combines sum(E_t * vc_shift) run DURING the
   q-conv: multiplies on Vector tensor_scalar (4x), pair adds on GpSimd.
 - after the last tap only: sumE, 1/sumE, and per output parity plane
   (F~ * r) * src -- ~14 us of Vector tail.
The v-conv runs first on the PE so vc exists before the fold prework.

Layout: host-packed polyphase f16 planes of the padded 129x129 grid;
65-wide planes carry a leading zero pad column (width 66) so fold-stage
reads are 4-byte aligned. A column-shifted aligned shadow of vc (vcs)
keeps the v01/v11 reads aligned. Outputs are four 64x64 parity planes in
f16, re-interleaved on the host.

Plane layouts (rows x cols, * = zero pad col):
  EE [65 x 66] = [* | P[0::2, 0::2]]   (leading pad)
  EO [65 x 64] =      P[0::2, 1::2]
  OE [64 x 66] = [* | P[1::2, 0::2]]   (leading pad)
  OO [64 x 64] =      P[1::2, 1::2]
where P is the zero-padded [129,129] grid, P[r,c] = x[r-1, c-1].
"""

import numpy as np

N_CORES = 8
SCALE = 1.0 / 64.0

# plane offsets within the packed polyphase layout
P_EE, P_EO, P_OE, P_OO = 0, 4290, 8450, 12674
SRCN = 16770                    # 65*66 + 65*64 + 64*66 + 64*64
FEATN = 4290                    # 65*66 natural padded grid (2 pad cols)
OUTN = 16384                    # 4 planes x 64*64

_prog_cache = {}
TRACE = False
TRACE_KW = {}
LAST_RESULT = [None]
STAGE = [99]

# matmul / copy chunks (first small so the PE starts early)
CHUNKS = [1024] + [2048] * 7 + [1410]
EE_RDY, EO_RDY, OE_RDY, OO_RDY = 2, 4, 6, 8


def _build(add_bv: bool, stage: int = 99):
    import concourse.mybir as mybir
    import concourse.tile as tile
    from concourse import bacc
    from concourse.dve_ops import TENSOR_TENSOR_REDUCE

    f32 = mybir.dt.float32
    f16 = mybir.dt.float16
    ADD = mybir.AluOpType.add
    MULT = mybir.AluOpType.mult
    AX = mybir.AxisListType.X
    Exp = mybir.ActivationFunctionType.Exp
    Copy = mybir.ActivationFunctionType.Copy

    nc = bacc.Bacc("TRN2", target_bir_lowering=False, debug=False,
                   num_devices=N_CORES)

    src_d = nc.dram_tensor("src", [256, SRCN], f16, kind="ExternalInput").ap()
    feat_d = nc.dram_tensor("feat", [256, FEATN], f16,
                            kind="ExternalInput").ap()
    wpack_d = nc.dram_tensor("wpack", [256, 256], f16,
                             kind="ExternalInput").ap()
    sinit_d = nc.dram_tensor("s_init", [128, 9], f32,
                             kind="ExternalInput").ap()
    bv_d = nc.dram_tensor("bv", [128, 1], f32, kind="ExternalInput").ap()
    out_d = nc.dram_tensor("out", [128, OUTN], f16, kind="ExternalOutput").ap()

    coff = [0]
    for cs in CHUNKS:
        coff.append(coff[-1] + cs)

    with tile.TileContext(nc) as tc:
        with (
            tc.tile_pool(name="constp", bufs=2) as constp,
            tc.tile_pool(name="srcp", bufs=2) as srcp,
            tc.tile_pool(name="featp", bufs=2) as featp,
            tc.tile_pool(name="qcp", bufs=1) as qcp,
            tc.tile_pool(name="vcp", bufs=1) as vcp,
            tc.tile_pool(name="smp", bufs=1) as smp,
            tc.tile_pool(name="mp", bufs=1) as mp,
            tc.tile_pool(name="outp", bufs=2) as outp,
            tc.tile_pool(name="ps", bufs=2, space="PSUM") as ps,
        ):
            # smalls (cols): [0:9] S  [9:18] E  [27:28] sumE  [28:29] r
            # [32:41] s_init  [48:49] bv
            sm = smp.tile([128, 64], f32, tag="smalls")
            nc.sync.dma_start(sm[:, 32:41], sinit_d[:, :])
            if add_bv:
                nc.sync.dma_start(sm[:, 48:49], bv_d[:, :])

            # ---- loads ----
            w_t = []
            for kt in range(2):
                wt = constp.tile([128, 256], f16, tag="w")
                nc.sync.dma_start(wt[:], wpack_d[128 * kt:128 * kt + 128, :])
                w_t.append(wt)
            src_t = [srcp.tile([128, SRCN], f16, tag="src", name=f"src{k}")
                     for k in range(2)]
            feat_t = [featp.tile([128, FEATN], f16, tag="feat",
                                 name=f"feat{k}") for k in range(2)]
            for kt in range(2):
                nc.sync.dma_start(src_t[kt][:, 0:coff[1]],
                                  src_d[128 * kt:128 * kt + 128, 0:coff[1]])
            nc.sync.dma_start(feat_t[0][:], feat_d[0:128, :])
            for kt in range(2):
                nc.sync.dma_start(
                    src_t[kt][:, coff[1]:coff[2]],
                    src_d[128 * kt:128 * kt + 128, coff[1]:coff[2]])
            nc.sync.dma_start(feat_t[1][:], feat_d[128:256, :])
            for c in range(2, len(CHUNKS)):
                for kt in range(2):
                    nc.sync.dma_start(
                        src_t[kt][:, coff[c]:coff[c + 1]],
                        src_d[128 * kt:128 * kt + 128, coff[c]:coff[c + 1]])

            qc = qcp.tile([128, SRCN], f16, tag="qc")
            qEE = qc[:, P_EE:P_EO].rearrange("p (r q) -> p r q", q=66)
            qEO = qc[:, P_EO:P_OE].rearrange("p (r q) -> p r q", q=64)
            qOE = qc[:, P_OE:P_OO].rearrange("p (r q) -> p r q", q=66)
            qOO = qc[:, P_OO:SRCN].rearrange("p (r q) -> p r q", q=64)
            kv = feat_t[0].rearrange("p (r q) -> p r q", q=66)[:, 0:64, 0:64]

            # fold-prework working tiles; mE/mF double as tap scratch
            mA = mp.tile([128, 4096], f16, tag="mA")
            mB = mp.tile([128, 4096], f16, tag="mB")
            mC = mp.tile([128, 4096], f16, tag="mC")
            mD = mp.tile([128, 4096], f16, tag="mD")
            mE = mp.tile([128, 4224], f16, tag="mE")
            mF = mp.tile([128, 4224], f16, tag="mF")

            def v2(tl):
                return tl.rearrange("p (r q) -> p r q", q=64)

            def e(t):
                return sm[:, 9 + t:10 + t]

            def tap_ttr(t, qview, scr):
                nc.vector._custom_dve(
                    TENSOR_TENSOR_REDUCE,
                    out=scr[:, 0:4096].rearrange("p (r q) -> p r q", q=64),
                    in0=qview, in1=kv, s0=0.0, s1=SCALE,
                    accum_out=sm[:, t:t + 1])
                nc.scalar.activation(e(t), sm[:, t:t + 1], Exp,
                                     bias=sm[:, 32 + t:33 + t], scale=1.0)

            def tap_split(t, qview, scr):
                s3 = scr[:, 0:4096].rearrange("p (r q) -> p r q", q=64)
                nc.vector.tensor_tensor(out=s3, in0=qview, in1=kv, op=MULT)
                nc.scalar.activation(scr[:, 0:4096], scr[:, 0:4096], Copy,
                                     bias=0.0, scale=SCALE,
                                     accum_out=sm[:, t:t + 1])
                nc.scalar.activation(e(t), sm[:, t:t + 1], Exp,
                                     bias=sm[:, 32 + t:33 + t], scale=1.0)

            def emit_vconv():
                vc = vcp.tile([128, FEATN], f16, tag="vc")
                for c0 in range(0, FEATN, 2048):
                    csz = min(2048, FEATN - c0)
                    pt = ps.tile([128, 2048], f32, tag="mm")
                    for kt in range(2):
                        for s0 in range(0, csz, 512):
                            ssz = min(512, csz - s0)
                            nc.tensor.matmul(
                                pt[:, s0:s0 + ssz],
                                lhsT=w_t[kt][:, 128:256],
                                rhs=feat_t[kt][:, c0 + s0:c0 + s0 + ssz],
                                start=(kt == 0), stop=(kt == 1))
                    if add_bv:
                        nc.vector.tensor_scalar(
                            out=vc[:, c0:c0 + csz], in0=pt[:, 0:csz],
                            scalar1=sm[:, 48:49], scalar2=None, op0=ADD)
                    else:
                        nc.scalar.copy(vc[:, c0:c0 + csz], pt[:, 0:csz])
                vc3 = vc.rearrange("p (r q) -> p r q", q=66)
                if add_bv:
                    nc.gpsimd.memset(vc3[:, 64, :], 0.0)
                    nc.gpsimd.memset(vc3[:, :, 64:66], 0.0)
                vcs = vcp.tile([128, 65 * 64], f16, tag="vcs")
                vcs3 = vcs.rearrange("p (r q) -> p r q", q=64)
                nc.vector.tensor_copy(vcs3[:], vc3[:, 0:65, 1:65])
                return vc3, vcs3

            vc3 = vcs3 = None
            views = {}
            for c, csz in enumerate(CHUNKS):
                c0 = coff[c]
                # v-conv + vcs after chunk 2's matmuls (feat arrived by then)
                if c == 3 and stage >= 4:
                    vc3, vcs3 = emit_vconv()
                    views = dict(v00=vc3[:, 0:64, 0:64],
                                 v10=vc3[:, 1:65, 0:64],
                                 v01=vcs3[:, 0:64, :],
                                 v11=vcs3[:, 1:65, :])
                pt = ps.tile([128, 2048], f32, tag="mm")
                for kt in range(2):
                    for s0 in range(0, csz, 512):
                        ssz = min(512, csz - s0)
                        nc.tensor.matmul(
                            pt[:, s0:s0 + ssz],
                            lhsT=w_t[kt][:, 0:128],
                            rhs=src_t[kt][:, c0 + s0:c0 + s0 + ssz],
                            start=(kt == 0), stop=(kt == 1))
                if stage < 2 and c > 0:
                    continue
                nc.scalar.copy(qc[:, c0:c0 + csz], pt[:, 0:csz])
                if stage < 2:
                    continue
                if c == 2:      # EE plane ready: taps t0,t6 (ttr) t2,t8 (split)
                    tap_ttr(0, qEE[:, 0:64, 1:65], mE)
                    tap_ttr(6, qEE[:, 1:65, 1:65], mE)
                    tap_split(2, qEE[:, 0:64, 2:66], mF)
                    tap_split(8, qEE[:, 1:65, 2:66], mE)
                if c == 3 and stage >= 5:
                    # oo prework: E{0,2,6,8} * v -> mA..mD; G pair-sums
                    nc.vector.tensor_scalar(out=v2(mA), in0=views['v11'],
                                            scalar1=e(0), scalar2=None,
                                            op0=MULT)
                    nc.vector.tensor_scalar(out=v2(mB), in0=views['v10'],
                                            scalar1=e(2), scalar2=None,
                                            op0=MULT)
                    nc.vector.tensor_scalar(out=v2(mC), in0=views['v01'],
                                            scalar1=e(6), scalar2=None,
                                            op0=MULT)
                    nc.vector.tensor_scalar(out=v2(mD), in0=views['v00'],
                                            scalar1=e(8), scalar2=None,
                                            op0=MULT)
                    nc.gpsimd.tensor_tensor(out=v2(mA), in0=v2(mA),
                                            in1=v2(mB), op=ADD)
                    nc.gpsimd.tensor_tensor(out=v2(mC), in0=v2(mC),
                                            in1=v2(mD), op=ADD)
                if c == 4:      # EO plane: t1 (ttr), t7 (split)
                    tap_ttr(1, qEO[:, 0:64, 0:64], mE)
                    tap_split(7, qEO[:, 1:65, 0:64], mF)
                    if stage >= 5:
                        # oe prework: mB = E1*v10, mD = E7*v00; G: mB += mD
                        nc.vector.tensor_scalar(out=v2(mB), in0=views['v10'],
                                                scalar1=e(1), scalar2=None,
                                                op0=MULT)
                        nc.vector.tensor_scalar(out=v2(mD), in0=views['v00'],
                                                scalar1=e(7), scalar2=None,
                                                op0=MULT)
                        nc.gpsimd.tensor_tensor(out=v2(mB), in0=v2(mB),
                                                in1=v2(mD), op=ADD)
                if c == 6:      # OE plane: t3, t5 (split)
                    if stage >= 5:
                        # finish oo: mA += mC (after G pair-sums)
                        nc.vector.tensor_tensor(out=v2(mA), in0=v2(mA),
                                                in1=v2(mC), op=ADD)
                    tap_split(3, qOE[:, 0:64, 1:65], mF)
                    tap_split(5, qOE[:, 0:64, 2:66], mE)
                    if stage >= 5:
                        # eo prework: mC = E3*v01, mD = E5*v00; G: mC += mD
                        nc.vector.tensor_scalar(out=v2(mC), in0=views['v01'],
                                                scalar1=e(3), scalar2=None,
                                                op0=MULT)
                        nc.vector.tensor_scalar(out=v2(mD), in0=views['v00'],
                                                scalar1=e(5), scalar2=None,
                                                op0=MULT)
                        nc.gpsimd.tensor_tensor(out=v2(mC), in0=v2(mC),
                                                in1=v2(mD), op=ADD)
                if c == 8:      # OO plane: t4 (ttr, the gate)
                    tap_ttr(4, qOO[:, 0:64, 0:64], mE)

            if stage == 2:
                nc.sync.dma_start(out_d[:, 0:9], sm[:, 0:9])

            # ---- normalization ----
            if stage >= 3:
                nc.vector.tensor_reduce(sm[:, 27:28], sm[:, 9:18],
                                        axis=AX, op=ADD)
                nc.vector.reciprocal(sm[:, 28:29], sm[:, 27:28])
            if stage == 3:
                nc.sync.dma_start(out_d[:, 16:25], sm[:, 9:18])

            # ---- tail: (F~ * r) * src per parity plane ----
            if stage >= 5:
                r = sm[:, 28:29]
                s3 = src_t[0]
                sEE = s3[:, P_EE:P_EO].rearrange("p (r q) -> p r q", q=66)
                sEO = s3[:, P_EO:P_OE].rearrange("p (r q) -> p r q", q=64)
                sOE = s3[:, P_OE:P_OO].rearrange("p (r q) -> p r q", q=66)
                sOO = s3[:, P_OO:SRCN].rearrange("p (r q) -> p r q", q=64)

                # ee: mD = (E4*v00)*r in one 2-scalar op (mD free)
                nc.vector.tensor_scalar(out=v2(mD), in0=views['v00'],
                                        scalar1=e(4), scalar2=r,
                                        op0=MULT, op1=MULT)
                Pee = outp.tile([128, 4096], f16, tag="O")
                nc.vector.tensor_tensor(out=v2(Pee), in0=v2(mD),
                                        in1=sOO[:, 0:64, 0:64], op=MULT)
                nc.sync.dma_start(out_d[:, 0:4096], Pee[:])
                # oo (F~oo in mA)
                nc.vector.tensor_scalar(out=v2(mA), in0=v2(mA), scalar1=r,
                                        scalar2=None, op0=MULT)
                Poo = outp.tile([128, 4096], f16, tag="O")
                nc.vector.tensor_tensor(out=v2(Poo), in0=v2(mA),
                                        in1=sEE[:, 1:65, 2:66], op=MULT)
                nc.sync.dma_start(out_d[:, 12288:16384], Poo[:])
                # oe (F~oe in mB) -> multiplies the EO src plane
                nc.vector.tensor_scalar(out=v2(mB), in0=v2(mB), scalar1=r,
                                        scalar2=None, op0=MULT)
                Poe = outp.tile([128, 4096], f16, tag="O")
                nc.vector.tensor_tensor(out=v2(Poe), in0=v2(mB),
                                        in1=sEO[:, 1:65, 0:64], op=MULT)
                nc.sync.dma_start(out_d[:, 8192:12288], Poe[:])
                # eo (F~eo in mC) -> multiplies the OE src plane
                nc.vector.tensor_scalar(out=v2(mC), in0=v2(mC), scalar1=r,
                                        scalar2=None, op0=MULT)
                Peo = outp.tile([128, 4096], f16, tag="O")
                nc.vector.tensor_tensor(out=v2(Peo), in0=v2(mC),
                                        in1=sOE[:, 0:64, 2:66], op=MULT)
                nc.sync.dma_start(out_d[:, 4096:8192], Peo[:])

    nc.compile()
    return nc


def _get_program(add_bv: bool, stage: int = 99):
    key = (add_bv, stage)
    if key not in _prog_cache:
        _prog_cache[key] = _build(add_bv, stage)
    return _prog_cache[key]


def _polyphase(x):
    # x: [B, C, 129, 129] padded f16 -> [B, C, 16770] plane-packed with
    # leading zero pad col on the 65-wide (even-col) planes
    B, C = x.shape[:2]
    ee = np.zeros((B, C, 65, 66), np.float16)
    ee[:, :, :, 1:66] = x[:, :, 0::2, 0::2]
    oe = np.zeros((B, C, 64, 66), np.float16)
    oe[:, :, :, 1:66] = x[:, :, 1::2, 0::2]
    return np.concatenate([
        ee.reshape(B, C, -1),
        x[:, :, 0::2, 1::2].reshape(B, C, -1),
        oe.reshape(B, C, -1),
        x[:, :, 1::2, 1::2].reshape(B, C, -1),
    ], axis=2)


def kernel(feat, src, Wq, bq, Wv, bv):
    from concourse.bass_utils import run_bass_kernel_spmd

    feat = np.asarray(feat, dtype=np.float32)
    src = np.asarray(src, dtype=np.float32)
    Wq = np.asarray(Wq, dtype=np.float32)
    bq = np.asarray(bq, dtype=np.float32)
    Wv = np.asarray(Wv, dtype=np.float32)
    bv = np.asarray(bv, dtype=np.float32)
    B, C, H, W = src.shape
    CH_HALF = C // 2

    # padded 129x129 grid (row/col -1 pad; right/bottom pad never read)
    src_pad = np.zeros((B, C, 129, 129), np.float16)
    src_pad[:, :, 1:129, 1:129] = src
    src_pk = _polyphase(src_pad)                       # [B, C, 16770]
    feat_pk = np.zeros((B, C, 65, 66), np.float16)
    feat_pk[:, :, 0:64, 0:64] = feat
    feat_pk = feat_pk.reshape(B, C, FEATN)

    add_bv = bool(np.any(bv))
    nc = _get_program(add_bv, STAGE[0])

    in_maps = []
    for core in range(N_CORES):
        b, u = core // 2, core % 2
        own = slice(CH_HALF * u, CH_HALF * u + CH_HALF)
        perm = np.r_[own, slice(CH_HALF * (1 - u), CH_HALF * (1 - u) + CH_HALF)]
        wpack = np.concatenate(
            [Wq[own][:, perm].T, Wv[own][:, perm].T], axis=1
        ).astype(np.float16)
        # bq correction seed: S += bq * sum(valid k) * scale; valid excludes
        # x=0 when i==0 and y=0 when j==0 (qc zero-pad positions).
        if np.any(bq):
            k = feat[b, own].astype(np.float64)
            tot = k.sum((1, 2))
            no_r0 = tot - k[:, 0, :].sum(1)
            no_c0 = tot - k[:, :, 0].sum(1)
            no_rc = no_r0 - k[:, :, 0].sum(1) + k[:, 0, 0]
            sums = [no_rc, no_r0, no_r0, no_c0, tot, tot, no_c0, tot, tot]
            sinit = (np.stack(sums, 1) * bq[own, None] * SCALE).astype(
                np.float32)
        else:
            sinit = np.zeros((CH_HALF, 9), np.float32)
        in_maps.append({
            "src": np.ascontiguousarray(src_pk[b, perm]),
            "feat": np.ascontiguousarray(feat_pk[b, perm]),
            "wpack": np.ascontiguousarray(wpack),
            "s_init": sinit,
            "bv": bv[own].reshape(CH_HALF, 1).astype(np.float32),
        })

    res = run_bass_kernel_spmd(nc, in_maps, list(range(N_CORES)),
                               trace=TRACE, **TRACE_KW)
    LAST_RESULT[0] = res

    out = np.empty((B, C, H, W), np.float32)
    for core in range(N_CORES):
        b, u = core // 2, core % 2
        own = slice(CH_HALF * u, CH_HALF * u + CH_HALF)
        r = res.results[core]["out"].astype(np.float32).reshape(
            CH_HALF, 4, 64, 64)
        out[b, own, 0::2, 0::2] = r[:, 0]
        out[b, own, 0::2, 1::2] = r[:, 1]
        out[b, own, 1::2, 0::2] = r[:, 2]
        out[b, own, 1::2, 1::2] = r[:, 3]
    return out
